# revision 5
# baseline (speedup 1.0000x reference)
"""ClipMatcher detection-loss kernel for 8 Trainium2 NeuronCores.

Strategy (data-parallel over frames, per the sharding hint):
  - 1920 frames split 8 x 240; each core processes its frames fully.
  - Phase A (anchor-gt IoU matching) is factorized: overlap widths depend
    only on (cx, shape) -> 192 values, heights on (cy, shape) -> 192, so
    inter = iw*ih via broadcast-view multiply.  Since iou = inter/(C-inter)
    is monotone in inter per shape (C = Aa_s + Ag + eps, only 12 distinct
    anchor areas), the per-frame max-iou and the mask threshold both reduce
    to comparisons on `inter` -- no full-width division:
      mask = inter >= tau_s,  tau_s = t_p * C_s/(1+t_p),
      t_p = min(0.2+, max_iou)   (== `iou>0.2 OR argmax` semantics).
  - BCE dense term: relu(l) + ln1p(exp(-|l|)) summed via fused ACT
    accumulators; masked correction sum(l*mask) on DVE.
  - Phase B (l1 + GIoU on refined boxes) in bf16 (error budget: the loss is
    dominated by the BCE term; l1/giou contribute ~0.25% of it); divisions
    via exp(ln a - ln b) on the Scalar engine (DVE reciprocal measured ~6x
    slower than a tensor op at full width).  Masked sums via fused
    accumulators.
  - Each core returns per-partition accumulator columns [128, 12]; final
    scalar reduction on host (the "all-reduce" is 8 x 12 x 128 floats).
"""

import numpy as np
import ml_dtypes

import concourse.bass as bass
import concourse.tile as tile
from concourse import mybir
from concourse.vector_clock import ScopedClock
from concourse.bass_utils import run_bass_kernel_spmd
from contextlib import ExitStack

# ----------------------------------------------------------------------------
# walrus workaround: this container's neuronxcc rejects instructions carrying
# more than one semaphore sync-wait; split extras onto single-wait NOPs.
# ----------------------------------------------------------------------------
_PATCHED = False


def _split_waits(nc, inst, add_nop):
    si = getattr(inst, "sync_info", None)
    if si is None or not si.on_wait or len(si.on_wait) <= 1:
        return
    eng = getattr(inst, "engine", None)
    if eng is None or eng == mybir.EngineType.Unassigned:
        return
    waits = list(si.on_wait)
    si.on_wait = [waits[-1]]
    for w in waits[:-1]:
        nop = mybir.InstNoOp(
            name=nc.get_next_instruction_name(),
            engine=eng,
            sync_info=mybir.SyncInfo(on_wait=[w], on_update=[]),
            bass_nofuse=True,
        )
        add_nop(nop)


def _apply_patches():
    global _PATCHED
    if _PATCHED:
        return
    _PATCHED = True

    _orig_tc_add = tile.TileContext._add_instruction

    def _tc_add(self, inst):
        _split_waits(self.nc, inst, lambda nop: _orig_tc_add(self, nop))
        return _orig_tc_add(self, inst)

    tile.TileContext._add_instruction = _tc_add

    _orig_bass_add = bass.Bass._add_instruction

    def _bass_add(self, ins, **kwargs):
        _split_waits(self, ins, lambda nop: _orig_bass_add(self, nop))
        return _orig_bass_add(self, ins, **kwargs)

    bass.Bass._add_instruction = _bass_add

    def _drain_and_barrier(self, tick_clock, wait_clock):
        drain_inst = self.nc.sync.drain()
        wait_clock.add_sem_waits(
            drain_inst.ins, ScopedClock({None: tick_clock.global_clock})
        )
        si = drain_inst.ins.sync_info
        waits = list(si.on_wait) if (si is not None and si.on_wait) else []
        if len(waits) > 1:
            si.on_wait = [waits[0]]
            for w in waits[1:]:
                nop = self.nc.sync.nop(nofuse=True, hint="split_tail_wait")
                nsi = nop.ins.sync_info
                if nsi is None:
                    nop.ins.sync_info = mybir.SyncInfo(on_wait=[w], on_update=[])
                else:
                    nsi.on_wait = [w]
        self.nc.all_engine_barrier()
        assert self.sems is not None
        popped = self.nc._tile_sem_poison_stack.pop()
        assert popped is self._sem_poison
        self.nc.clear_and_free_semaphores(list(self.sems.allocated().values()))
        self.nc.all_engine_barrier()

    tile.TileContext._drain_and_barrier = _drain_and_barrier


# ----------------------------------------------------------------------------
# problem constants (hardcoded per contract)
# ----------------------------------------------------------------------------
BT, N = 1920, 3072
NCORES = 8
FPC = BT // NCORES            # 240 frames per core
TILE_PS = [128, FPC - 128]    # frame-tile partition counts [128, 112]
NCHUNK = 4
CW = N // NCHUNK              # anchor chunk width 768
POS_THR = 0.2
EPS = 1e-7
W_GIOU = 0.3
W_PROB = 100.0

F32 = mybir.dt.float32
BF16 = mybir.dt.bfloat16
A = mybir.AluOpType
AF = mybir.ActivationFunctionType

# accumulator slot map (columns of the [128, 12] output)
SLOT_NPOS = (0, 1)            # per frame-tile
SLOT_LN1P = (2, 3)
SLOT_RELU = (4, 5)
SLOT_LM = (6, 7)
SLOT_V = ((8, 9, 10, 11), (12, 13, 14, 15))   # [tile][chunk]
NSLOT = 16

# G-param columns
GX2, NGX1, GY2, NGY1, NHGCX, NHGCY, NGWH, NGHH, AG, AGE = range(10)

_STATE = {}


def _fview(t, P, dims, offset_extra=0):
    """View of SBUF tile `t` with partition count P and custom free dims."""
    return bass.AP(
        tensor=t.tensor,
        offset=t.offset + offset_extra,
        ap=[[t.ap[0][0], P]] + [list(d) for d in dims],
    )


def _build_program(debug_taps=False, reps=1):
    _apply_patches()
    nc = bass.Bass("TRN2", target_bir_lowering=False, debug=False)

    pred_d = nc.dram_tensor("pred", [FPC, N * 4], F32, kind="ExternalInput")
    cls_d = nc.dram_tensor("cls", [FPC, N], F32, kind="ExternalInput")
    g_d = nc.dram_tensor("gparams", [FPC, 10], F32, kind="ExternalInput")
    ax2_d = nc.dram_tensor("ax2c", [128, 192], F32, kind="ExternalInput")
    nax1_d = nc.dram_tensor("nax1c", [128, 192], F32, kind="ExternalInput")
    ay2_d = nc.dram_tensor("ay2c", [128, 192], F32, kind="ExternalInput")
    nay1_d = nc.dram_tensor("nay1c", [128, 192], F32, kind="ExternalInput")
    aa12_d = nc.dram_tensor("aa12c", [128, 12], F32, kind="ExternalInput")
    acx_d = nc.dram_tensor("acxc", [128, 192], BF16, kind="ExternalInput")
    acy_d = nc.dram_tensor("acyc", [128, 16], BF16, kind="ExternalInput")
    awh_d = nc.dram_tensor("awhc", [128, 12], BF16, kind="ExternalInput")
    ahh_d = nc.dram_tensor("ahhc", [128, 12], BF16, kind="ExternalInput")
    acc_d = nc.dram_tensor("acc", [128, NSLOT], F32, kind="ExternalOutput")
    taps = {}
    if debug_taps:
        taps["inter0"] = nc.dram_tensor("inter0", [128, N], F32, kind="ExternalOutput")
        taps["mask0"] = nc.dram_tensor("mask0", [128, N], F32, kind="ExternalOutput")
        taps["gio0"] = nc.dram_tensor("gio0", [128, CW], F32, kind="ExternalOutput")
        taps["u0"] = nc.dram_tensor("u0", [128, CW], F32, kind="ExternalOutput")

    THRP = float(np.nextafter(np.float32(POS_THR), np.float32(1.0)))
    SAFE = float(np.float32(1.0) - np.float32(2.0 ** -20))

    with tile.TileContext(nc) as tc:
        with ExitStack() as ctx:
            consts = ctx.enter_context(tc.tile_pool(name="consts", bufs=1))
            io = ctx.enter_context(tc.tile_pool(name="io", bufs=2))
            ph_a = ctx.enter_context(tc.tile_pool(name="ph_a", bufs=1))
            maskp = ctx.enter_context(tc.tile_pool(name="maskp", bufs=2))
            ph_b = ctx.enter_context(tc.tile_pool(name="ph_b", bufs=1))
            small = ctx.enter_context(tc.tile_pool(name="small", bufs=2))
            accp = ctx.enter_context(tc.tile_pool(name="accp", bufs=1))

            acc = accp.tile([128, NSLOT], F32)
            nc.vector.memset(acc, 0.0)
            epsc = accp.tile([128, 1], F32)
            nc.vector.memset(epsc, EPS)

            ax2 = consts.tile([128, 192], F32)
            nax1 = consts.tile([128, 192], F32)
            ay2 = consts.tile([128, 192], F32)
            nay1 = consts.tile([128, 192], F32)
            aa12 = consts.tile([128, 12], F32)
            acx = consts.tile([128, 192], BF16)
            acy = consts.tile([128, 16], BF16)
            awh = consts.tile([128, 12], BF16)
            ahh = consts.tile([128, 12], BF16)
            for dst, src in [(ax2, ax2_d), (nax1, nax1_d), (ay2, ay2_d),
                             (nay1, nay1_d), (aa12, aa12_d), (acx, acx_d),
                             (acy, acy_d), (awh, awh_d), (ahh, ahh_d)]:
                nc.sync.dma_start(out=dst, in_=src.ap())

            pred_ap = pred_d.ap()
            cls_ap = cls_d.ap()
            g_ap = g_d.ap()

            for rep in range(reps):
              t0 = 0
              for ti, P in enumerate(TILE_PS):
                G = io.tile([128, 10], F32, tag="G")
                nc.sync.dma_start(out=G[:P], in_=g_ap[t0:t0 + P])
                CLS = io.tile([128, N], F32, tag="CLS")
                nc.sync.dma_start(out=CLS[:P], in_=cls_ap[t0:t0 + P])

                def gcol(c, P=P, G=G):
                    return G[:P, c:c + 1]

                # ---------------- phase A ----------------
                a1 = small.tile([128, 192], F32, tag="a1")
                a2 = small.tile([128, 192], F32, tag="a2")
                iwf = small.tile([128, 192], F32, tag="iwf")
                ihf = small.tile([128, 192], F32, tag="ihf")
                nc.vector.tensor_scalar(out=a1[:P], in0=ax2[:P], scalar1=gcol(GX2),
                                        scalar2=None, op0=A.min)
                nc.vector.tensor_scalar(out=a2[:P], in0=nax1[:P], scalar1=gcol(NGX1),
                                        scalar2=None, op0=A.min)
                nc.vector.tensor_tensor(out=a1[:P], in0=a1[:P], in1=a2[:P], op=A.add)
                nc.vector.tensor_scalar(out=iwf[:P], in0=a1[:P], scalar1=0.0,
                                        scalar2=None, op0=A.max)
                nc.vector.tensor_scalar(out=a1[:P], in0=ay2[:P], scalar1=gcol(GY2),
                                        scalar2=None, op0=A.min)
                nc.vector.tensor_scalar(out=a2[:P], in0=nay1[:P], scalar1=gcol(NGY1),
                                        scalar2=None, op0=A.min)
                nc.vector.tensor_tensor(out=a1[:P], in0=a1[:P], in1=a2[:P], op=A.add)
                nc.vector.tensor_scalar(out=ihf[:P], in0=a1[:P], scalar1=0.0,
                                        scalar2=None, op0=A.max)

                inter = ph_a.tile([128, N], F32, tag="inter")
                iw_v = _fview(iwf, P, [[0, 16], [1, 192]])
                ih_v = _fview(ihf, P, [[12, 16], [0, 16], [1, 12]])
                nc.vector.tensor_tensor(out=inter[:P], in0=iw_v, in1=ih_v, op=A.mult)

                # per-(frame, shape) max of inter: view [P, 12(s), 256(pos)]
                m12 = small.tile([128, 12], F32, tag="m12")
                inter_sv = _fview(inter, P, [[1, 12], [12, 256]])
                nc.vector.tensor_reduce(op=A.max, out=m12[:P], in_=inter_sv,
                                        axis=mybir.AxisListType.X)
                c12 = small.tile([128, 12], F32, tag="c12")
                nc.vector.tensor_scalar(out=c12[:P], in0=aa12[:P], scalar1=gcol(AGE),
                                        scalar2=None, op0=A.add)
                d12 = small.tile([128, 12], F32, tag="d12")
                nc.vector.tensor_tensor(out=d12[:P], in0=c12[:P], in1=m12[:P],
                                        op=A.subtract)
                nc.vector.reciprocal(out=d12[:P], in_=d12[:P])
                nc.vector.tensor_tensor(out=d12[:P], in0=m12[:P], in1=d12[:P],
                                        op=A.mult)
                mx = small.tile([128, 1], F32, tag="mx")
                nc.vector.tensor_reduce(op=A.max, out=mx[:P], in_=d12[:P],
                                        axis=mybir.AxisListType.X)
                tp = small.tile([128, 1], F32, tag="tp")
                nc.vector.tensor_scalar(out=tp[:P], in0=mx[:P], scalar1=THRP,
                                        scalar2=None, op0=A.min)
                tp1 = small.tile([128, 1], F32, tag="tp1")
                nc.vector.tensor_scalar(out=tp1[:P], in0=tp[:P], scalar1=1.0,
                                        scalar2=None, op0=A.add)
                nc.vector.reciprocal(out=tp1[:P], in_=tp1[:P])
                nc.vector.tensor_tensor(out=tp[:P], in0=tp[:P], in1=tp1[:P],
                                        op=A.mult)
                nc.vector.tensor_scalar(out=tp[:P], in0=tp[:P], scalar1=SAFE,
                                        scalar2=None, op0=A.mult)
                tau = small.tile([128, 12], F32, tag="tau")
                nc.vector.tensor_scalar(out=tau[:P], in0=c12[:P],
                                        scalar1=tp[:P, 0:1], scalar2=None,
                                        op0=A.mult)

                maskb = maskp.tile([128, N], BF16, tag="maskb")
                tau_v = _fview(tau, P, [[0, 256], [1, 12]])
                nc.vector.tensor_tensor(out=maskb[:P], in0=inter[:P], in1=tau_v,
                                        op=A.is_ge)
                scr_a = ph_a.tile([128, N], BF16, tag="scr_a")
                nc.vector.tensor_scalar(
                    out=scr_a[:P], in0=maskb[:P], scalar1=1.0, scalar2=None,
                    op0=A.mult, op1=A.add,
                    accum_out=acc[:P, SLOT_NPOS[ti]:SLOT_NPOS[ti] + 1])
                if debug_taps and ti == 0:
                    nc.sync.dma_start(out=taps["inter0"].ap(), in_=inter)
                    mf = ph_a.tile([128, N], F32, tag="mf")
                    nc.vector.tensor_copy(mf[:P], maskb[:P])
                    nc.sync.dma_start(out=taps["mask0"].ap(), in_=mf)

                # ---------------- BCE dense ----------------
                s1 = ph_a.tile([128, N], F32, tag="s1")
                s2 = ph_a.tile([128, N], F32, tag="s2")
                nc.scalar.activation(s1[:P], CLS[:P], AF.Abs)
                nc.scalar.activation(s2[:P], s1[:P], AF.Exp, scale=-1.0)
                nc.scalar.activation(
                    s1[:P], s2[:P], AF.Ln, bias=1.0,
                    accum_out=acc[:P, SLOT_LN1P[ti]:SLOT_LN1P[ti] + 1])
                nc.scalar.activation(
                    s2[:P], CLS[:P], AF.Relu,
                    accum_out=acc[:P, SLOT_RELU[ti]:SLOT_RELU[ti] + 1])
                clsb = ph_a.tile([128, N], BF16, tag="clsb")
                nc.vector.tensor_copy(clsb[:P], CLS[:P])
                lmb = ph_a.tile([128, N], BF16, tag="lmb")
                nc.vector.tensor_tensor(out=lmb[:P], in0=clsb[:P], in1=maskb[:P],
                                        op=A.mult)
                nc.vector.tensor_scalar(
                    out=scr_a[:P], in0=lmb[:P], scalar1=1.0, scalar2=None,
                    op0=A.mult, op1=A.add,
                    accum_out=acc[:P, SLOT_LM[ti]:SLOT_LM[ti] + 1])

                # ---------------- phase B (bf16), per anchor chunk ----------
                for k in range(NCHUNK):
                    R4 = io.tile([128, 4 * CW], F32, tag="R4")
                    nc.sync.dma_start(
                        out=R4[:P],
                        in_=pred_ap[t0:t0 + P, k * 4 * CW:(k + 1) * 4 * CW])

                    def comp(c, P=P, R4=R4):
                        return _fview(R4, P, [[4, CW]], offset_extra=c)

                    cyo = k * CW // 192
                    acx_v = _fview(acx, P, [[0, CW // 192], [1, 192]])
                    acy_v = _fview(acy, P, [[1, CW // 192], [0, 192]],
                                   offset_extra=cyo)
                    awh_v = _fview(awh, P, [[0, CW // 12], [1, 12]])
                    ahh_v = _fview(ahh, P, [[0, CW // 12], [1, 12]])

                    # refined box (center / half-extent), bf16
                    bcx = ph_b.tile([128, CW], BF16, tag="bcx")
                    bcy = ph_b.tile([128, CW], BF16, tag="bcy")
                    hwx = ph_b.tile([128, CW], BF16, tag="hwx")
                    hwy = ph_b.tile([128, CW], BF16, tag="hwy")
                    nc.vector.tensor_tensor(out=bcx[:P], in0=comp(0), in1=acx_v,
                                            op=A.add)
                    nc.vector.tensor_tensor(out=bcy[:P], in0=comp(1), in1=acy_v,
                                            op=A.add)
                    e1 = ph_b.tile([128, CW], BF16, tag="e1")
                    nc.scalar.activation(e1[:P], comp(2), AF.Copy, scale=0.5)
                    nc.vector.tensor_tensor(out=hwx[:P], in0=e1[:P], in1=awh_v,
                                            op=A.add)
                    nc.scalar.activation(e1[:P], comp(3), AF.Copy, scale=0.5)
                    nc.vector.tensor_tensor(out=hwy[:P], in0=e1[:P], in1=ahh_v,
                                            op=A.add)

                    # xyxy pieces
                    px2 = ph_b.tile([128, CW], BF16, tag="px2")
                    nx1 = ph_b.tile([128, CW], BF16, tag="nx1")
                    py2 = ph_b.tile([128, CW], BF16, tag="py2")
                    ny1 = ph_b.tile([128, CW], BF16, tag="ny1")
                    nc.vector.tensor_tensor(out=px2[:P], in0=bcx[:P], in1=hwx[:P],
                                            op=A.add)
                    nc.vector.tensor_tensor(out=nx1[:P], in0=hwx[:P], in1=bcx[:P],
                                            op=A.subtract)
                    nc.vector.tensor_tensor(out=py2[:P], in0=bcy[:P], in1=hwy[:P],
                                            op=A.add)
                    nc.vector.tensor_tensor(out=ny1[:P], in0=hwy[:P], in1=bcy[:P],
                                            op=A.subtract)

                    # l1 terms now (kills bcx/bcy); scale/bias bake the 0.5
                    u = ph_b.tile([128, CW], BF16, tag="u")
                    t1 = ph_b.tile([128, CW], BF16, tag="t1")
                    nc.scalar.activation(u[:P], bcx[:P], AF.Abs, bias=gcol(NHGCX),
                                         scale=0.5)
                    nc.scalar.activation(t1[:P], bcy[:P], AF.Abs, bias=gcol(NHGCY),
                                         scale=0.5)
                    nc.vector.tensor_tensor(out=u[:P], in0=u[:P], in1=t1[:P], op=A.add)
                    nc.scalar.activation(t1[:P], hwx[:P], AF.Abs, bias=gcol(NGWH))
                    nc.vector.tensor_tensor(out=u[:P], in0=u[:P], in1=t1[:P], op=A.add)
                    nc.scalar.activation(t1[:P], hwy[:P], AF.Abs, bias=gcol(NGHH))
                    nc.vector.tensor_tensor(out=u[:P], in0=u[:P], in1=t1[:P], op=A.add)

                    # pred area quarter: relu(hwx)*relu(hwy)  (kills hwx/hwy)
                    ap4 = ph_b.tile([128, CW], BF16, tag="ap4")
                    nc.vector.tensor_scalar(out=t1[:P], in0=hwx[:P], scalar1=0.0,
                                            scalar2=None, op0=A.max)
                    nc.vector.tensor_scalar(out=ap4[:P], in0=hwy[:P], scalar1=0.0,
                                            scalar2=None, op0=A.max)
                    nc.vector.tensor_tensor(out=ap4[:P], in0=ap4[:P], in1=t1[:P],
                                            op=A.mult)

                    # intersection (reuse bcx/bcy/hwx/hwy slots via tags t2/t3)
                    t2 = ph_b.tile([128, CW], BF16, tag="t2")
                    ib = ph_b.tile([128, CW], BF16, tag="ib")
                    nc.vector.tensor_scalar(out=t1[:P], in0=px2[:P],
                                            scalar1=gcol(GX2), scalar2=None, op0=A.min)
                    nc.vector.tensor_scalar(out=t2[:P], in0=nx1[:P],
                                            scalar1=gcol(NGX1), scalar2=None, op0=A.min)
                    nc.vector.tensor_tensor(out=t1[:P], in0=t1[:P], in1=t2[:P], op=A.add)
                    nc.vector.tensor_scalar(out=t1[:P], in0=t1[:P], scalar1=0.0,
                                            scalar2=None, op0=A.max)
                    nc.vector.tensor_scalar(out=t2[:P], in0=py2[:P],
                                            scalar1=gcol(GY2), scalar2=None, op0=A.min)
                    nc.vector.tensor_scalar(out=ib[:P], in0=ny1[:P],
                                            scalar1=gcol(NGY1), scalar2=None, op0=A.min)
                    nc.vector.tensor_tensor(out=t2[:P], in0=t2[:P], in1=ib[:P], op=A.add)
                    nc.vector.tensor_scalar(out=t2[:P], in0=t2[:P], scalar1=0.0,
                                            scalar2=None, op0=A.max)
                    nc.vector.tensor_tensor(out=ib[:P], in0=t1[:P], in1=t2[:P], op=A.mult)

                    # enclosure (kills px2/nx1/py2/ny1)
                    enc = ph_b.tile([128, CW], BF16, tag="enc")
                    nc.vector.tensor_scalar(out=t1[:P], in0=px2[:P],
                                            scalar1=gcol(GX2), scalar2=None, op0=A.max)
                    nc.vector.tensor_scalar(out=t2[:P], in0=nx1[:P],
                                            scalar1=gcol(NGX1), scalar2=None, op0=A.max)
                    nc.vector.tensor_tensor(out=t1[:P], in0=t1[:P], in1=t2[:P], op=A.add)
                    nc.vector.tensor_scalar(out=t2[:P], in0=py2[:P],
                                            scalar1=gcol(GY2), scalar2=None, op0=A.max)
                    nc.vector.tensor_scalar(out=enc[:P], in0=ny1[:P],
                                            scalar1=gcol(NGY1), scalar2=None, op0=A.max)
                    nc.vector.tensor_tensor(out=t2[:P], in0=t2[:P], in1=enc[:P], op=A.add)
                    nc.vector.tensor_tensor(out=enc[:P], in0=t1[:P], in1=t2[:P], op=A.mult)

                    # union = 4*ap4 + Ag - inter
                    U = ph_b.tile([128, CW], BF16, tag="U")
                    nc.vector.tensor_scalar(out=t1[:P], in0=ap4[:P], scalar1=4.0,
                                            scalar2=gcol(AG), op0=A.mult, op1=A.add)
                    nc.vector.tensor_tensor(out=U[:P], in0=t1[:P], in1=ib[:P],
                                            op=A.subtract)

                    # giou = exp(ln ib - ln(U+eps)) - exp(ln relu(enc-U) - ln(enc+eps))
                    nc.scalar.activation(t1[:P], ib[:P], AF.Ln)
                    nc.scalar.activation(t2[:P], U[:P], AF.Ln, bias=epsc[:P, 0:1])
                    nc.vector.tensor_tensor(out=t1[:P], in0=t1[:P], in1=t2[:P],
                                            op=A.subtract)
                    iou = ph_b.tile([128, CW], BF16, tag="iou")
                    nc.scalar.activation(iou[:P], t1[:P], AF.Exp)
                    nc.vector.tensor_tensor(out=t1[:P], in0=enc[:P], in1=U[:P],
                                            op=A.subtract)
                    nc.vector.tensor_scalar(out=t1[:P], in0=t1[:P], scalar1=0.0,
                                            scalar2=None, op0=A.max)
                    nc.scalar.activation(t1[:P], t1[:P], AF.Ln)
                    nc.scalar.activation(t2[:P], enc[:P], AF.Ln, bias=epsc[:P, 0:1])
                    nc.vector.tensor_tensor(out=t1[:P], in0=t1[:P], in1=t2[:P],
                                            op=A.subtract)
                    nc.scalar.activation(t2[:P], t1[:P], AF.Exp)
                    gio = ph_b.tile([128, CW], BF16, tag="gio")
                    nc.vector.tensor_tensor(out=gio[:P], in0=iou[:P], in1=t2[:P],
                                            op=A.subtract)
                    if debug_taps and ti == 0 and k == 0:
                        gf = ph_b.tile([128, CW], F32, tag="gf")
                        nc.vector.tensor_copy(gf[:P], gio[:P])
                        nc.sync.dma_start(out=taps["gio0"].ap(), in_=gf)
                        nc.vector.tensor_copy(gf[:P], u[:P])
                        nc.sync.dma_start(out=taps["u0"].ap(), in_=gf)

                    # V = u - 0.3*giou; masked sum -> slot
                    nc.vector.tensor_scalar(out=gio[:P], in0=gio[:P],
                                            scalar1=-W_GIOU, scalar2=None, op0=A.mult)
                    nc.vector.tensor_tensor(out=u[:P], in0=u[:P], in1=gio[:P],
                                            op=A.add)
                    nc.vector.tensor_tensor(out=u[:P], in0=u[:P],
                                            in1=maskb[:P, k * CW:(k + 1) * CW],
                                            op=A.mult)
                    sv = SLOT_V[ti][k]
                    nc.vector.tensor_scalar(out=t1[:P], in0=u[:P], scalar1=1.0,
                                            scalar2=None, op0=A.mult, op1=A.add,
                                            accum_out=acc[:P, sv:sv + 1])

                t0 += P

            nc.sync.dma_start(out=acc_d.ap(), in_=acc)

    return nc


def _prep_consts(anchors):
    a = np.asarray(anchors, dtype=np.float32).reshape(16, 16, 12, 4)
    acx16 = a[0, :, 0, 0]          # center x by cx
    acy16 = a[:, 0, 0, 1]          # center y by cy
    aw12 = a[0, 0, :, 2]
    ah12 = a[0, 0, :, 3]
    ax2 = (acx16[:, None] + aw12[None, :] / 2).reshape(-1)      # [192] cx*12+s
    nax1 = (aw12[None, :] / 2 - acx16[:, None]).reshape(-1)
    ay2 = (acy16[:, None] + ah12[None, :] / 2).reshape(-1)
    nay1 = (ah12[None, :] / 2 - acy16[:, None]).reshape(-1)
    aa12 = aw12 * ah12
    acx192 = np.repeat(acx16, 12)

    def bc(v, dt=np.float32):
        v = np.asarray(v, dtype=np.float32)
        return np.broadcast_to(v.astype(dt), (128, v.shape[0])).copy()

    bf = ml_dtypes.bfloat16
    return {
        "ax2c": bc(ax2), "nax1c": bc(nax1), "ay2c": bc(ay2), "nay1c": bc(nay1),
        "aa12c": bc(aa12),
        "acxc": bc(acx192, bf), "acyc": bc(acy16, bf),
        "awhc": bc(aw12 / 2, bf), "ahhc": bc(ah12 / 2, bf),
    }


def _prep_gparams(gt):
    g = np.asarray(gt, dtype=np.float32)
    gcx, gcy, gw, gh = g[:, 0], g[:, 1], g[:, 2], g[:, 3]
    return np.stack([
        gcx + gw / 2,            # GX2
        gw / 2 - gcx,            # NGX1 = -gx1
        gcy + gh / 2,            # GY2
        gh / 2 - gcy,            # NGY1
        -gcx / 2,                # NHGCX
        -gcy / 2,                # NHGCY
        -gw / 2,                 # NGWH
        -gh / 2,                 # NGHH
        gw * gh,                 # AG
        gw * gh + EPS,           # AGE
    ], axis=1).astype(np.float32)


def make_in_maps(pred_reg, pred_cls, gt_xyhw, anchors_xyhw):
    pred_reg = np.ascontiguousarray(np.asarray(pred_reg, dtype=np.float32))
    pred_cls = np.ascontiguousarray(np.asarray(pred_cls, dtype=np.float32))
    consts = _prep_consts(anchors_xyhw)
    gparams = _prep_gparams(gt_xyhw)
    in_maps = []
    for c in range(NCORES):
        s = slice(c * FPC, (c + 1) * FPC)
        in_maps.append({
            "pred": pred_reg[s].reshape(FPC, N * 4),
            "cls": pred_cls[s].reshape(FPC, N),
            "gparams": gparams[s],
            **consts,
        })
    return in_maps


def finalize(acc_list):
    tot = np.zeros(NSLOT, dtype=np.float64)
    for a in acc_list:
        tot += np.asarray(a, dtype=np.float64).sum(axis=0)
    npos_tot = tot[SLOT_NPOS[0]] + tot[SLOT_NPOS[1]]
    s_ln1p = tot[SLOT_LN1P[0]] + tot[SLOT_LN1P[1]]
    s_relu = tot[SLOT_RELU[0]] + tot[SLOT_RELU[1]]
    s_lm = tot[SLOT_LM[0]] + tot[SLOT_LM[1]]
    s_v = sum(tot[s] for pair in SLOT_V for s in pair)
    npos_c = max(npos_tot, 1.0)
    loss_pos = (s_v + W_GIOU * npos_tot) / npos_c
    loss_prob = (s_relu + s_ln1p - s_lm) / float(BT * N)
    return np.float32(loss_pos + W_PROB * loss_prob)


def _get_program():
    if "nc" not in _STATE:
        _STATE["nc"] = _build_program()
    return _STATE["nc"]


def kernel(pred_reg, pred_cls, gt_xyhw, anchors_xyhw):
    nc = _get_program()
    in_maps = make_in_maps(pred_reg, pred_cls, gt_xyhw, anchors_xyhw)
    res = run_bass_kernel_spmd(nc, in_maps, core_ids=list(range(NCORES)))
    return finalize([res.results[c]["acc"] for c in range(NCORES)])
